# revision 10
# baseline (speedup 1.0000x reference)
"""Trainium2 Bass kernel for the Boat Dynamic System — rank-8 CP, fp16, v7.

Math: out[b, c] = s~^T Q_c s~ with s~ = (1, u, v, r, Pf); pro/rud folded on
host. The 4x5x5 tensor {Q_c} is CP-decomposed (Gauss-Newton, exact fit) as
Q_c = sum_{j=0..7} lam[c,j] w_j w_j^T, then W is rounded to fp16 and
(lam, bias) are REFIT in f64 so quantization cancels to first order.

v7 balances the elementwise work across ACT and DVE (the only two engines
with a PSUM read port; DVE has ONE such port so tensor ops may read at most
one PSUM operand) and moves every DMA to HWDGE (sync/scalar queues):
  - ACT: Square(y + b) for 13 of the 16 square-chunks (its only func ->
    the activation table never reloads, 1.3us each)
  - DVE: for chunks in DVE_ZB, the B-half square runs two-stage:
    t = fp16(yB + bB) via tensor_scalar (PSUM->SBUF), then zB = t*t via
    tensor_tensor (all-SBUF fp16 -> 2x DVE mode)
  - DVE: all 8 M2-output PSUM->SBUF fp16 casts (tensor_copy)

Per [128, 1024] chunk (8 per core): 2 fp16 matmuls wA -> yA psum, 2 wB ->
yB, ACT/DVE squares -> fp16 SBUF, then (pipelined one chunk behind) 4
accumulating lam matmuls back into yA's banks, DVE cast, sync HWDGE DMA
out per chunk. Host does all layout permutes + f32 upcast.
"""

import os

import numpy as np

NCORES = 8
B = 2097152
BS = B // NCORES          # 262144 rows per core
DT = 0.01
NTILES = 4                # [128, 2048] tiles per core
TILE_F = 2048             # free dim per tile (512 batch x 4 comps)
NCHUNK = 512              # matmul free size (one PSUM f32 bank)
CPT = TILE_F // NCHUNK    # 4 chunks per tile
NFUNC = 8

_NC_CACHE = {}
LAST_RESULT = [None]

# ---------------------------------------------------------------- host math

_MONO_QUAD = [(0, 0), (0, 1), (0, 2), (0, 3), (1, 1), (1, 2), (1, 3),
              (2, 2), (2, 3), (3, 3)]


def _build_Q(t, cmd, coeffs):
    idx = int(np.round(float(np.asarray(t).reshape(-1)[0]) / DT))
    pro = float(cmd[idx, 0])
    rud = float(cmd[idx, 1])
    cf = np.asarray(coeffs, dtype=np.float64)
    ceff = cf[:, 0:15] + pro * cf[:, 15:30] + rud * cf[:, 30:45]  # [4,15]

    Q = np.zeros((4, 5, 5))
    Q[:, 0, 0] = ceff[:, 0]
    for f in range(4):
        Q[:, 0, 1 + f] += ceff[:, 1 + f] / 2
        Q[:, 1 + f, 0] += ceff[:, 1 + f] / 2
    for k, (x, y) in enumerate(_MONO_QUAD):
        m = 5 + k
        if x == y:
            Q[:, 1 + x, 1 + x] += ceff[:, m]
        else:
            Q[:, 1 + x, 1 + y] += ceff[:, m] / 2
            Q[:, 1 + y, 1 + x] += ceff[:, m] / 2
    return Q


def _cp_decompose(Q, N=NFUNC, restarts=40, iters=200, seed=0, tol=1e-11):
    """Gauss-Newton (LM) exact symmetric CP fit: Q_c = sum_j lam_cj w_j w_j^T."""
    rng = np.random.default_rng(seed)
    qn = np.linalg.norm(Q)
    best = None
    iu = np.triu_indices(5)
    wts = np.where(iu[0] == iu[1], 1.0, np.sqrt(2.0))
    a_idx, b_idx = iu

    def resid(W, lam):
        outer = np.einsum('ja,jb->jab', W, W)
        R = Q - np.einsum('cj,jab->cab', lam, outer)
        return (R[:, a_idx, b_idx] * wts).ravel()

    NJ = N * 5 + 4 * N
    for trial in range(restarts):
        W = rng.standard_normal((N, 5))
        lam = rng.standard_normal((4, N)) * 0.1
        mu = 1e-6
        for _ in range(iters):
            r = resid(W, lam)
            res = np.linalg.norm(r)
            J = np.zeros((60, NJ))
            for j in range(N):
                for e in range(5):
                    contrib = (np.where(a_idx == e, W[j, b_idx], 0.0)
                               + np.where(b_idx == e, W[j, a_idx], 0.0)) * wts
                    J[:, j * 5 + e] = (-lam[:, j][:, None]
                                       * contrib[None, :]).ravel()
            outerj = W[:, a_idx] * W[:, b_idx] * wts
            for c in range(4):
                for j in range(N):
                    col = np.zeros((4, 15))
                    col[c] = -outerj[j]
                    J[:, N * 5 + c * N + j] = col.ravel()
            JTJ = J.T @ J
            g = J.T @ r
            ok = False
            for _ in range(40):
                try:
                    step = np.linalg.solve(JTJ + mu * np.eye(NJ), -g)
                except np.linalg.LinAlgError:
                    mu *= 10
                    continue
                Wn = W + step[:N * 5].reshape(N, 5)
                lamn = lam + step[N * 5:].reshape(4, N)
                if np.linalg.norm(resid(Wn, lamn)) < res:
                    W, lam = Wn, lamn
                    mu = max(mu / 3, 1e-12)
                    ok = True
                    break
                mu *= 10
                if mu > 1e12:
                    break
            if not ok:
                break
            if res < tol * qn * 0.1:
                break
        res = np.linalg.norm(resid(W, lam)) / qn
        if res < tol:
            amp = float(np.abs(lam).sum(axis=1).max() * (np.abs(W).max() ** 2))
            if best is None or amp < best[3]:
                best = (W.copy(), lam.copy(), res, amp)
            if trial >= 2 and best[3] < 60:
                break
    if best is None:
        raise RuntimeError("CP decomposition failed to converge")
    return best


def _refit_fp16(Q, W, lam):
    """Round W[:,1:5] to fp16; refit (lam, bias) in f64; quantize lam to
    fp16 with a final bias refit."""
    N = W.shape[0]
    iu = np.triu_indices(5)
    wts = np.where(iu[0] == iu[1], 1.0, np.sqrt(2.0))
    Tq = Q[:, iu[0], iu[1]] * wts                     # [4,15]

    Wf = W.astype(np.float64).copy()
    Wf[:, 1:5] = Wf[:, 1:5].astype(np.float16)
    b = Wf[:, 0].copy()

    def gram(Wfull):
        return np.einsum('ja,jb->jab', Wfull, Wfull)[:, iu[0], iu[1]] * wts

    def fit_lam(Wfull):
        G = gram(Wfull)
        lam2, *_ = np.linalg.lstsq(G.T, Tq.T, rcond=None)
        return lam2.T, G

    def refit_bias(lamx, b):
        for _ in range(100):
            Wf[:, 0] = b
            G = gram(Wf)
            R = Tq - lamx @ G
            J = np.zeros((60, N))
            for j in range(N):
                dG = np.zeros((5, 5))
                dG[0, :] += Wf[j]
                dG[:, 0] += Wf[j]
                J[:, j] = np.outer(lamx[:, j], dG[iu[0], iu[1]] * wts).ravel()
            step, *_ = np.linalg.lstsq(J, R.ravel(), rcond=None)
            b = b + step
            if np.linalg.norm(step) < 1e-13:
                break
        return b

    for _ in range(3):
        Wf[:, 0] = b
        lam2, _ = fit_lam(Wf)
        b = refit_bias(lam2, b)
    lam16 = lam2.astype(np.float16).astype(np.float64)
    b = refit_bias(lam16, b)
    Wf[:, 0] = b
    return Wf, b, lam16


def _host_weights(t, cmd, coeffs):
    """Device weights: 4x [128,128] block-diag kron mats (fp16) + biases.

    biasp columns: 0 = bA (A-half Square bias), 1 = bB (B-half bias).
    """
    Q = _build_Q(t, cmd, coeffs)
    W, lam, res, amp = _cp_decompose(Q)
    Wf, bias, lam16 = _refit_fp16(Q, W, lam)

    I32 = np.eye(32)
    wA = np.kron(I32, Wf[0:4, 1:5].T).astype(np.float16)    # [128,128]
    wB = np.kron(I32, Wf[4:8, 1:5].T).astype(np.float16)
    lamA = np.kron(I32, lam16[:, 0:4].T).astype(np.float16)
    lamB = np.kron(I32, lam16[:, 4:8].T).astype(np.float16)
    biasp = np.stack([
        np.tile(bias[0:4], 32),
        np.tile(bias[4:8], 32),
    ], axis=1).astype(np.float32)                           # [128, 2]
    return wA, wB, lamA, lamB, biasp


# ---------------------------------------------------------------- device

def _build_nc():
    import concourse.bacc as bacc
    import concourse.mybir as mybir
    import concourse.tile as tile

    nc = bacc.Bacc("TRN2", target_bir_lowering=False, debug=False)
    f32 = mybir.dt.float32
    f16 = mybir.dt.float16
    Square = mybir.ActivationFunctionType.Square
    Alu = mybir.AluOpType

    # [q, (T g)]: per tile, 4KB contiguous per partition
    xt_d = nc.dram_tensor("xt", [128, NTILES * TILE_F], f16,
                          kind="ExternalInput")
    wpack_d = nc.dram_tensor("wpack", [128, 512], f16, kind="ExternalInput")
    biasp_d = nc.dram_tensor("biasp", [128, 2], f32, kind="ExternalInput")
    out = nc.dram_tensor("out", [128, NTILES * TILE_F], f16,
                         kind="ExternalOutput")

    NH = NTILES * TILE_F // 1024          # 8 chunks of 1024 cols

    with tile.TileContext(nc) as tc:
        with (
            tc.tile_pool(name="consts", bufs=1) as cpool,
            tc.tile_pool(name="xt", bufs=4) as xtp,
            tc.tile_pool(name="z", bufs=2) as zp,
            tc.tile_pool(name="onat", bufs=4) as onp_,
            tc.tile_pool(name="pa", bufs=2, space="PSUM") as pap,
            tc.tile_pool(name="pb", bufs=2, space="PSUM") as pbp,
        ):
            wpack = cpool.tile([128, 512], f16)
            biasp = cpool.tile([128, 2], f32)
            warm = cpool.tile([128, 1], f32)
            # weights on the ACT HWDGE ring; x / out on the sync ring so the
            # two streams start in parallel
            nc.scalar.dma_start(out=wpack[:], in_=wpack_d[:, :])
            nc.scalar.dma_start(out=biasp[:], in_=biasp_d[:, :])
            # load the Square ACT table set during the DMA head, off the
            # critical path (the only ACT func used -> no table reloads)
            nc.scalar.activation(out=warm[:], in_=biasp[:, 0:1], func=Square,
                                 bias=0.0, scale=1.0)
            wA = wpack[:, 0:128]
            wB = wpack[:, 128:256]
            lamA = wpack[:, 256:384]
            lamB = wpack[:, 384:512]

            xTs = []
            for T in range(NTILES):
                xT = xtp.tile([128, TILE_F], f16, tag=f"xt{T}",
                              name=f"xt{T}")
                xTs.append(xT)
            # first chunk split out so compute starts after 256 KiB
            nc.sync.dma_start(out=xTs[0][:, 0:1024], in_=xt_d[:, 0:1024])
            nc.sync.dma_start(out=xTs[0][:, 1024:2048],
                              in_=xt_d[:, 1024:2048])
            for T in range(1, NTILES):
                nc.sync.dma_start(
                    out=xTs[T][:], in_=xt_d[:, T * TILE_F:(T + 1) * TILE_F]
                )
            onats = [onp_.tile([128, TILE_F], f16, tag=f"onat{i}",
                               name=f"onat{i}") for i in range(NTILES)]

            DVE_ZB = {0, 4, 7}        # chunks whose B-half squares on DVE

            def m2_block(st):
                """M2 (into the yA tile, WAR after ACT) + DVE cast +
                per-chunk sync HWDGE DMA out."""
                h, yA, zA, zB = st
                for u in range(2):
                    nc.tensor.matmul(
                        out=yA[:, u * NCHUNK:(u + 1) * NCHUNK],
                        lhsT=lamA,
                        rhs=zA[:, u * NCHUNK:(u + 1) * NCHUNK],
                        start=True, stop=False,
                    )
                for u in range(2):
                    nc.tensor.matmul(
                        out=yA[:, u * NCHUNK:(u + 1) * NCHUNK],
                        lhsT=lamB,
                        rhs=zB[:, u * NCHUNK:(u + 1) * NCHUNK],
                        start=False, stop=True,
                    )
                T, c0 = h // 2, (h % 2) * 1024
                nc.vector.tensor_copy(
                    out=onats[T][:, c0:c0 + 1024], in_=yA[:]
                )
                s0 = h * 1024
                nc.sync.dma_start(
                    out=out[:, s0:s0 + 1024],
                    in_=onats[T][:, c0:c0 + 1024],
                )

            prev = None
            for h in range(NH):           # chunk = 1024 cols = 2 matmuls
                xT = xTs[h // 2]
                c0 = (h % 2) * 1024
                yA = pap.tile([128, 2 * NCHUNK], f32)
                yB = pbp.tile([128, 2 * NCHUNK], f32)
                for u in range(2):
                    nc.tensor.matmul(
                        out=yA[:, u * NCHUNK:(u + 1) * NCHUNK],
                        lhsT=wA,
                        rhs=xT[:, c0 + u * NCHUNK:c0 + (u + 1) * NCHUNK],
                        start=True, stop=True,
                    )
                for u in range(2):
                    nc.tensor.matmul(
                        out=yB[:, u * NCHUNK:(u + 1) * NCHUNK],
                        lhsT=wB,
                        rhs=xT[:, c0 + u * NCHUNK:c0 + (u + 1) * NCHUNK],
                        start=True, stop=True,
                    )
                zA = zp.tile([128, 2 * NCHUNK], f16, tag="zA")
                zB = zp.tile([128, 2 * NCHUNK], f16, tag="zB")
                nc.scalar.activation(out=zA[:], in_=yA[:], func=Square,
                                     bias=biasp[:, 0:1], scale=1.0)
                if h in DVE_ZB:
                    tB = zp.tile([128, 2 * NCHUNK], f16, tag="tB")
                    nc.vector.tensor_scalar(
                        out=tB[:], in0=yB[:], scalar1=biasp[:, 1:2],
                        scalar2=None, op0=Alu.add,
                    )
                    nc.vector.tensor_tensor(
                        out=zB[:], in0=tB[:], in1=tB[:], op=Alu.mult,
                    )
                else:
                    nc.scalar.activation(out=zB[:], in_=yB[:], func=Square,
                                         bias=biasp[:, 1:2], scale=1.0)
                if prev is not None:
                    m2_block(prev)
                prev = (h, yA, zA, zB)
            m2_block(prev)

    nc.finalize()
    return nc


def _ensure_ntff_hook():
    """Install the axon NTFF profiling hook if the image's antenv lacks it."""
    import sys
    import types
    try:
        from antenv.axon_hooks import get_axon_ntff_profile_hook  # noqa: F401
        return
    except ImportError:
        pass
    try:
        import antenv
        from trn_agent_boot.trn_boot import _ntff_profile_via_ctypes
        mod = types.ModuleType("antenv.axon_hooks")
        store = [None]
        mod.set_axon_ntff_profile_hook = lambda h: store.__setitem__(0, h)
        mod.get_axon_ntff_profile_hook = lambda: store[0]
        sys.modules["antenv.axon_hooks"] = mod
        antenv.axon_hooks = mod
        mod.set_axon_ntff_profile_hook(
            _ntff_profile_via_ctypes("/opt/axon/libaxon_pjrt.so")
        )
        import concourse.bass_utils as bu
        bu.upload_artifacts = lambda tmpdir: tmpdir
    except Exception as e:  # profiling is best-effort
        print(f"ntff hook install failed: {e}")


def kernel(t, state, cmd, coeffs):
    from concourse.bass_utils import run_bass_kernel_spmd

    trace = bool(int(os.environ.get("BOAT_TRACE", "0")))
    if trace:
        _ensure_ntff_hook()

    t = np.asarray(t)
    state16 = np.asarray(state, dtype=np.float16)
    cmd = np.asarray(cmd, dtype=np.float32)
    coeffs = np.asarray(coeffs, dtype=np.float32)

    wA, wB, lamA, lamB, biasp = _host_weights(t, cmd, coeffs)
    wpack = np.concatenate([wA, wB, lamA, lamB], axis=1)   # [128, 512] f16

    if "nc" not in _NC_CACHE:
        _NC_CACHE["nc"] = _build_nc()
    nc = _NC_CACHE["nc"]

    in_maps = []
    for k in range(NCORES):
        shard = state16[k * BS:(k + 1) * BS]
        # xt[32bp + 4nh + f, T*2048 + 32bc + j] =
        #    state[(T*128 + 32bp + j)*512 + 8bc + nh, f]
        xt = np.ascontiguousarray(
            shard.reshape(NTILES, 4, 32, 64, 8, 4)      # T bp j bc nh f
            .transpose(1, 4, 5, 0, 3, 2)                # bp nh f T bc j
            .reshape(128, NTILES * TILE_F)
        )
        in_maps.append({"xt": xt, "wpack": wpack, "biasp": biasp})

    res = run_bass_kernel_spmd(
        nc,
        in_maps,
        core_ids=list(range(NCORES)),
        trace=trace,
    )
    LAST_RESULT[0] = res
    outs = []
    for r in res.results:
        # out[(T*128+32bp+j)*512 + 8bc + nh, c] =
        #    o[32bp + 4nh + c, T*2048 + 32bc + j]
        o = (r["out"].reshape(4, 8, 4, NTILES, 64, 32)   # bp nh c T bc j
             .transpose(3, 0, 5, 4, 1, 2)                # T bp j bc nh c
             .reshape(BS, 4))
        outs.append(o)
    return np.concatenate(outs, axis=0).astype(np.float32)

